# revision 29
# baseline (speedup 1.0000x reference)
"""Trainium2 Bass kernel for nn_MessageFunction (GNN message passing).

Math (reference):
  a_in[b,i,d]  = sum_j (matrix_in [adj[b,i,j]] @ h[b,j])[d]
  a_out[b,i,d] = sum_j (matrix_out[adj[b,j,i]] @ h[b,j])[d]
  out = concat([a_in, a_out], -1) + bias          # [B, N, 2D]

Strategy:
  - Data parallel: B=16 batches over 8 cores (2 per core).
  - One-hot over E=8 edge classes re-expressed in the *step basis*
    step_e(a) = 1[a >= e]:  onehot_e = step_e - step_{e+1}.  The host folds
    the basis change into the weights.  step_0 == all-ones contributes the
    rank-1 term (u0 @ sum_j h[j,:]) * ones[i]; that reduction is folded
    into a per-batch bias vector on the host (it is ~0.01% of the FLOPs),
    so the device handles only e = 1..7 -> 7 compare planes/orientation.
  - Per-class transformed states t[j, (dir,e,d)] = h @ Wt on the PE (bf16).
  - Aggregation computed transposed: a^T[d, i] = sum_e sum_j t_e[j,d] *
    plane_e[j,i] as accumulating bf16 matmuls (t chunk stationary, mask
    plane moving).  Two concurrent col-tiled matmuls (tile_position (0,0)
    and (0,64)) fill psum partitions 0:64 (a_in^T) and 64:128 (a_out^T).
  - Mask planes: DVE tensor_scalar is_ge (4x mode) + a few planes on the
    scalar engine as Sign activations (+-1 valued; the host halves those
    weight columns and shifts the rank-1 bias term to compensate).  The
    ACT-plane set differs per batch parity (per-parity Wt variants) to
    balance DVE and ACT.
  - Bias (incl. rank-1 term) fused into the final PSUM->SBUF copy
    (scalar.add with a per-partition bias vector).  Host transposes
    [d,i] -> [i,d] on the way out.
"""

import numpy as np
import ml_dtypes

import concourse.bass as bass
import concourse.tile as tile
from concourse import bacc, mybir
from concourse import bass_utils

BF16 = ml_dtypes.bfloat16

B, N, D, E = 16, 512, 64, 8
NCORES = 8
BPC = B // NCORES          # batches per core
NT = N // 128              # j chunks (4)
TWO_D = 2 * D              # 128
EC = E - 1                 # device-side edge classes (e = 1..7)
WCOL = 2 * EC * D          # Wt columns per batch variant (896)

# Planes computed on the scalar engine as Sign activations, per batch
# parity: list indexed by b in range(BPC) of sets of (orient, e).
# orient 0 = "in" (planes from adjT), orient 1 = "out" (planes from adj).
# Classes with NO oriented entry here are computed as one double-width DVE
# is_ge op covering both orientations at once.
ACT_PLANES_B = [set(), set()]

# Benchmark-only knob: hoist mask-plane production out of the timing loop
# (output becomes garbage; used to attribute loop time to plane production).
_STATIC_PLANES = False


def _build_program(loop_n=None):
    """Build the per-core Bass/Tile program (identical on all 8 cores).

    loop_n: if set, wrap the whole body in tc.For_i(loop_n) (benchmarking
    only — repeats the same computation in one device execution).
    """
    nc = bacc.Bacc(
        "TRN2",
        target_bir_lowering=False,
        debug=False,
        enable_asserts=False,
        num_devices=1,
    )
    dt = mybir.dt

    # DRAM I/O.  adj+adjT are pre-tiled on host to [BPC, 128, 2*NT*512]
    # (adj in cols 0:2048, adjT in 2048:4096, free index jc*512 + i) so one
    # DMA per batch reads 8KB contiguous per partition.  hT and wt are
    # likewise merged into one [D, 512+896] tensor per batch.
    adj2_d = nc.dram_tensor("adj2", [BPC, 128, 2 * NT * N], dt.bfloat16,
                            kind="ExternalInput")
    hw_d = nc.dram_tensor("hw", [BPC, D, N + WCOL], dt.bfloat16, kind="ExternalInput")
    bias_d = nc.dram_tensor("bias", [TWO_D, BPC], dt.float32, kind="ExternalInput")
    out_d = nc.dram_tensor("out", [BPC, TWO_D, N], dt.float32, kind="ExternalOutput")

    with tile.TileContext(nc) as tc:
        with (
            tc.tile_pool(name="const", bufs=1) as const_pool,
            tc.tile_pool(name="adj2", bufs=2) as adj2_pool,
            tc.tile_pool(name="hw", bufs=2) as hw_pool,
            tc.tile_pool(name="plane", bufs=10) as plane_pool,
            tc.tile_pool(name="plane1", bufs=4) as plane1_pool,
            tc.tile_pool(name="tsb", bufs=2) as t_pool,
            tc.tile_pool(name="outsb", bufs=2) as out_pool,
            tc.tile_pool(name="psum_t", bufs=3, space="PSUM") as psum_t_pool,
            tc.tile_pool(name="psum_agg", bufs=2, space="PSUM") as psum_agg_pool,
        ):
            bias_sb = const_pool.tile([TWO_D, BPC], dt.float32, tag="bias")
            nc.gpsimd.dma_start(bias_sb[:], bias_d.ap()[:, :])
            # Per-e bias columns for Sign-activation planes: -(e - 0.5)
            actbias_sb = const_pool.tile([128, E], dt.float32, tag="actbias")
            for e in range(1, E):
                nc.gpsimd.memset(actbias_sb[:, e:e + 1], -(e - 0.5))
            # PE warm-up operands + ACT function-table pin (keeps the
            # 1283ns LoadActFuncSet out of the timed loop body).
            warm_sb = const_pool.tile([128, N], dt.bfloat16, tag="warm")
            nc.vector.memset(warm_sb[:], 0.0)
            actpin_sb = const_pool.tile([128, 1], dt.bfloat16, tag="actpin")
            nc.scalar.activation(actpin_sb[:], warm_sb[:, 0:1],
                                 mybir.ActivationFunctionType.Sign,
                                 bias=actbias_sb[:, 1:2], scale=1.0)

            static_planes = None
            if _STATIC_PLANES:
                sp = const_pool.tile([128, 2 * NT * N], dt.bfloat16, tag="spl")
                nc.vector.memset(sp[:], 1.0)
                static_planes = [sp[:, 0:NT * N], sp[:, NT * N:2 * NT * N]]

            def full_body(_iv=None):
              # input DMAs up front: adj2-b0 first (it gates the DVE plane
              # chain, the longest co-pipeline), then hw-b0, then b1.
              adj2_sbs = [adj2_pool.tile([128, 2 * NT * N], dt.bfloat16,
                                         name=f"adj2_{b}", tag="adj2")
                          for b in range(BPC)]
              hw_sbs = [hw_pool.tile([D, N + WCOL], dt.bfloat16,
                                     name=f"hw_{b}", tag="hw")
                        for b in range(BPC)]
              nc.sync.dma_start(adj2_sbs[0][:, 0:NT * N],
                                adj2_d.ap()[0][:, 0:NT * N])
              nc.sync.dma_start(hw_sbs[0][:], hw_d.ap()[0])
              nc.sync.dma_start(adj2_sbs[0][:, NT * N:2 * NT * N],
                                adj2_d.ap()[0][:, NT * N:2 * NT * N])
              nc.sync.dma_start(adj2_sbs[1][:], adj2_d.ap()[1])
              nc.sync.dma_start(hw_sbs[1][:], hw_d.ap()[1])
              # PE warm-up during the DMA fill (p-state ramp)
              psum_warm = psum_agg_pool.tile([128, N], dt.float32, tag="agg")
              for _ in range(3):
                  nc.tensor.matmul(psum_warm[:], lhsT=warm_sb[:, 0:128],
                                   rhs=warm_sb[:], start=True, stop=True)
              # ---- t = h @ Wt for BOTH batches first: the PE has fill
              # work while the DVE produces the first mask planes ----
              t_sbs = [t_pool.tile([128, NT * WCOL], dt.bfloat16,
                                   name=f"t_{b}", tag="tsb")
                       for b in range(BPC)]
              for b in range(BPC):
                hT_sb = hw_sbs[b][:, 0:N]
                wt_sb = hw_sbs[b][:, N:N + WCOL]
                for jc in range(NT):
                    psum_t = psum_t_pool.tile([128, WCOL], dt.float32, tag="pt")
                    for lo, hi in ((0, 512), (512, WCOL)):
                        nc.tensor.matmul(
                            psum_t[:, lo:hi],
                            lhsT=hT_sb[:, jc * 128:(jc + 1) * 128],
                            rhs=wt_sb[:, lo:hi],
                            start=True,
                            stop=True,
                        )
                    nc.scalar.copy(t_sbs[b][:, jc * WCOL:(jc + 1) * WCOL],
                                   psum_t[:])

              for b in range(BPC):
                act_set = ACT_PLANES_B[b]
                adj2_sb = adj2_sbs[b]
                adj_sb = adj2_sb[:, 0:NT * N]
                adjT_sb = adj2_sb[:, NT * N:2 * NT * N]
                t_sb = t_sbs[b]
                psum_agg = psum_agg_pool.tile([128, N], dt.float32, tag="agg")

                def t_slice(e, jc, orient):
                    lo = jc * WCOL + orient * (EC * D) + (e - 1) * D
                    return t_sb[:, lo:lo + D]

                # ---- mask planes + aggregation matmuls, e = 1..7 ----
                # Slow ACT Sign planes are scheduled second (not first: they
                # would stall the pipeline start; not last: they would stall
                # the final ACT out-copy).
                is_act = lambda e: any((o, e) in act_set for o in range(2))
                dve_es = [e for e in range(1, E) if not is_act(e)]
                act_es = [e for e in range(1, E) if is_act(e)]
                e_order = dve_es[:1] + act_es + dve_es[1:]
                for ei, e in enumerate(e_order):
                    if _STATIC_PLANES:
                        planes = [static_planes[0], static_planes[1]]
                    elif b == 0 and ei == 0 and not is_act(e):
                        # b0's first class as two singles: the out-plane
                        # needs only the first-DMA'd adj half, so the DVE
                        # starts before half B lands.
                        pl_out = plane1_pool.tile([128, NT * N], dt.bfloat16,
                                                  tag="plane1")
                        nc.vector.tensor_scalar(
                            pl_out[:], adj_sb, float(e), None,
                            op0=mybir.AluOpType.is_ge)
                        pl_in = plane1_pool.tile([128, NT * N], dt.bfloat16,
                                                 tag="plane1")
                        nc.vector.tensor_scalar(
                            pl_in[:], adjT_sb, float(e), None,
                            op0=mybir.AluOpType.is_ge)
                        planes = [pl_in, pl_out]
                    elif not is_act(e):
                        # one double-width DVE op -> both orientations' planes
                        pl2 = plane_pool.tile([128, 2 * NT * N], dt.bfloat16,
                                              tag="plane")
                        nc.vector.tensor_scalar(
                            pl2[:], adj2_sb[:], float(e), None,
                            op0=mybir.AluOpType.is_ge,
                        )
                        planes = [pl2[:, NT * N:2 * NT * N], pl2[:, 0:NT * N]]
                    else:
                        planes = []
                        for orient in range(2):  # 0 = in (adjT), 1 = out (adj)
                            src = adjT_sb if orient == 0 else adj_sb
                            pl = plane1_pool.tile([128, NT * N], dt.bfloat16,
                                                  tag="plane1")
                            if (orient, e) in act_set:
                                # sign(a - (e - 0.5)) in {-1, +1}
                                nc.scalar.activation(
                                    pl[:], src[:],
                                    mybir.ActivationFunctionType.Sign,
                                    bias=actbias_sb[:, e:e + 1], scale=1.0,
                                )
                            else:
                                nc.vector.tensor_scalar(
                                    pl[:], src[:], float(e), None,
                                    op0=mybir.AluOpType.is_ge,
                                )
                            planes.append(pl)
                    for jc in range(NT):
                        first = (ei == 0 and jc == 0)
                        last = (ei == EC - 1 and jc == NT - 1)
                        for orient in range(2):
                            nc.tensor.matmul(
                                psum_agg[orient * D:(orient + 1) * D, :],
                                lhsT=t_slice(e, jc, orient),
                                rhs=planes[orient][:, jc * N:(jc + 1) * N],
                                start=first,
                                stop=last,
                                tile_position=(0, orient * D),
                                skip_group_check=True,
                            )

                # ---- bias (incl. host-folded rank-1 term) + store ----
                out_sb = out_pool.tile([TWO_D, N], dt.float32, tag="outsb")
                nc.scalar.add(out_sb[:], psum_agg[:], bias_sb[:, b:b + 1])
                nc.sync.dma_start(out_d.ap()[b], out_sb[:])

            if loop_n is None:
                full_body()
            else:
                with tc.For_i(0, loop_n, 1,
                              hint_engines=(mybir.EngineType.PE,
                                            mybir.EngineType.DVE,
                                            mybir.EngineType.Activation)) as iv:
                    full_body(iv)

    nc.compile()
    return nc


def _prep_host_inputs(node_state, adj_mat, matrix_in, matrix_out, bias):
    """Host-side preprocessing: sharding, dtype casts, step-basis weights."""
    node_state = np.asarray(node_state, dtype=np.float32)
    adj_mat = np.asarray(adj_mat)
    matrix_in = np.asarray(matrix_in, dtype=np.float64)
    matrix_out = np.asarray(matrix_out, dtype=np.float64)
    bias = np.asarray(bias, dtype=np.float64)

    # Step-basis weights: u[0] = M[0]; u[e] = M[e] - M[e-1]
    def step_weights(M):
        u = np.empty_like(M)
        u[0] = M[0]
        u[1:] = M[1:] - M[:-1]
        return u

    u = [step_weights(matrix_in), step_weights(matrix_out)]  # dir 0 = in, 1 = out

    # Per batch parity: ACT planes are sign-valued (+-1 = 2*step - 1): halve
    # those weight columns; the other half joins the rank-1 (e=0) term.
    wt = np.empty((BPC, D, WCOL), dtype=np.float64)
    u0_eff = []                         # [b][dir] -> [D, D]
    for b in range(BPC):
        act_set = ACT_PLANES_B[b]
        u0b = [u[0][0].copy(), u[1][0].copy()]
        for dir_ in range(2):
            for e in range(1, E):
                c = u[dir_][e]
                if (dir_, e) in act_set:
                    c = 0.5 * c
                    u0b[dir_] = u0b[dir_] + c
                wt[b, :, dir_ * EC * D + (e - 1) * D:
                         dir_ * EC * D + e * D] = c.T
        u0_eff.append(u0b)
    wt = wt.astype(BF16)

    # Rank-1 (all-ones plane) term per batch, folded into the bias:
    #   r[dir][d] = sum_k u0_eff[dir][d,k] * (sum_j h[b,j,k])
    hsum = node_state.astype(np.float64).sum(axis=1)          # [B, D]
    bias_full = np.empty((B, TWO_D), dtype=np.float64)
    for gb in range(B):
        b = gb % BPC
        bias_full[gb, :D] = bias[:D] + u0_eff[b][0] @ hsum[gb]
        bias_full[gb, D:] = bias[D:] + u0_eff[b][1] @ hsum[gb]
    bias_full = bias_full.astype(np.float32)

    # Per-core shards
    adj_bf = adj_mat.astype(BF16)                      # [B, N, N]
    adjT_bf = np.ascontiguousarray(adj_mat.transpose(0, 2, 1)).astype(BF16)
    hT_bf = np.ascontiguousarray(node_state.transpose(0, 2, 1)).astype(BF16)  # [B,D,N]

    def tile_adj(x):  # [BPC, N, N] -> [BPC, 128, NT*N] with free (jc, i)
        return x.reshape(BPC, NT, 128, N).transpose(0, 2, 1, 3).reshape(BPC, 128, NT * N)

    in_maps = []
    for c in range(NCORES):
        sl = slice(c * BPC, (c + 1) * BPC)
        hw = np.concatenate([hT_bf[sl], wt], axis=2)
        adj2 = np.concatenate([tile_adj(adj_bf[sl]), tile_adj(adjT_bf[sl])], axis=2)
        in_maps.append({
            "adj2": np.ascontiguousarray(adj2),
            "hw": np.ascontiguousarray(hw),
            "bias": np.ascontiguousarray(bias_full[sl].T),   # [128, BPC]
        })
    return in_maps


_CACHED_NC = None


def get_program():
    global _CACHED_NC
    if _CACHED_NC is None:
        _CACHED_NC = _build_program()
    return _CACHED_NC


def run_on_cores(in_maps, **kwargs):
    nc = get_program()
    return bass_utils.run_bass_kernel_spmd(
        nc, in_maps, core_ids=list(range(NCORES)), **kwargs
    )


def kernel(node_state, adj_mat, matrix_in, matrix_out, bias):
    in_maps = _prep_host_inputs(node_state, adj_mat, matrix_in, matrix_out, bias)
    res = run_on_cores(in_maps)
    # Gather: each core returns out [BPC, 2D, N] (transposed layout)
    parts = []
    for c in range(NCORES):
        o = np.asarray(res.results[c]["out"])          # [BPC, 128, 512]
        parts.append(o.transpose(0, 2, 1))             # [BPC, N, 2D]
    return np.ascontiguousarray(np.concatenate(parts, axis=0).astype(np.float32))

